# revision 10
# baseline (speedup 1.0000x reference)
"""Multi-head causal attention on 8 Trainium2 NeuronCores.

Sharding: data-parallel over batch (4) x tensor-parallel over heads (2 groups
of 8 heads). Each core computes a partial output [T, C] for one batch element
using its 8 heads; the host sums the two partials per batch element (the
"all-reduce after out_proj" done during unshard).

Per-core algorithm (all layouts chosen so no on-device transposes are needed):
  inputs: xT [C, T] (x[b] transposed on host), Wq/Wk/Wv [C, 512], Wo [512, C],
          causal multiplicative masks [4, 128, 512] (bf16).
  QT = Wq^T @ x^T * 1/sqrt(Dh)  [512, T]   (lhsT = Wq chunk, rhs = xT chunk)
  KT = Wk^T @ x^T               [512, T]
  V  = x @ Wv                   [T, 512]   (lhsT = xT chunk, rhs = Wv)
       stored ones-augmented as V_aug [T, 8 heads, 65] bf16 (col 64 = 1.0)
  per head h, query-chunk j (512 wide), key-block kb (128 wide, causal only):
     sT  = K_h[kb]^T @ Q_h[:, j]           [128, 512] PSUM  (fp32r matmul)
     p   = exp(sT)                         bf16 (skip-max softmax: |s| < ~9)
     p  *= mask                            (diagonal blocks only)
     av += V_aug[kb, h]^T @ p              [65, 512] PSUM; row 64 = denom
  attn_outT[h, :, j] = av[:64] * (1/denom broadcast via ones-matmul)
  out[tb] = attn_outT[:, tb]^T @ Wo        [128, 1024] -> partial output
"""

import numpy as np
import ml_dtypes

import concourse.bass as bass
import concourse.bacc as bacc
import concourse.mybir as mybir
import concourse.tile as tile
from concourse import bass_utils

F32 = mybir.dt.float32
F32R = mybir.dt.float32r
BF16 = mybir.dt.bfloat16

B, T, C = 4, 2048, 1024
H, Dh = 16, 64
G = 2                 # head groups (tensor parallel)
HPG = H // G          # heads per group
GC = HPG * Dh         # group channels = 512
N_CORES = 8
TC = 512              # token chunk (phase 2 and query chunks)
KB = 128              # key block
N_TC = T // TC        # 4
N_KB = T // KB        # 16
N_CC = C // 128       # contraction chunks over C = 8
N_GCB = GC // 128     # chan blocks in a group = 4


def build_program():
    nc = bacc.Bacc("TRN2", target_bir_lowering=False, debug=False)

    xT = nc.dram_tensor("xT", [C, T], F32R, kind="ExternalInput").ap()
    wq = nc.dram_tensor("wq", [C, GC], F32R, kind="ExternalInput").ap()
    wk = nc.dram_tensor("wk", [C, GC], F32R, kind="ExternalInput").ap()
    wv = nc.dram_tensor("wv", [C, GC], F32R, kind="ExternalInput").ap()
    wo = nc.dram_tensor("wo", [GC, C], F32R, kind="ExternalInput").ap()
    masks = nc.dram_tensor("masks", [4, KB, TC], BF16, kind="ExternalInput").ap()
    ones_in = nc.dram_tensor("ones", [1, Dh], F32R, kind="ExternalInput").ap()
    out = nc.dram_tensor("out", [T, C], F32, kind="ExternalOutput").ap()

    with tile.TileContext(nc) as tc:
        with tc.tile_pool(name="persist", bufs=1) as pp:
            qt = pp.tile([128, N_GCB, T], F32R)        # QT (chan%128, chan//128, tok)
            kt = pp.tile([128, N_GCB, T], F32R)
            vaug = pp.tile([128, N_KB, HPG, Dh + 1], BF16)
            aot = pp.tile([128, N_GCB, T], F32R)       # attn_outT
            msk = pp.tile([128, 4, TC], BF16)
            ones = pp.tile([1, Dh], F32R)

            nc.sync.dma_start(msk[:], masks.rearrange("m p n -> p m n"))
            nc.sync.dma_start(ones[:], ones_in)
            nc.vector.memset(vaug[:, :, :, Dh:], 1.0)

            # ---------------- phase 2: qkv projections -----------------
            with (
                tc.tile_pool(name="wq_pool", bufs=1) as wqp,
                tc.tile_pool(name="x_pool", bufs=2) as xp,
                tc.tile_pool(name="proj_psum", bufs=4, space="PSUM") as pjp,
            ):
                wqs = wqp.tile([128, N_CC, GC], F32R, tag="wq")
                wks = wqp.tile([128, N_CC, GC], F32R, tag="wk")
                wvs = wqp.tile([128, N_CC, GC], F32R, tag="wv")
                nc.sync.dma_start(wqs[:], wq.rearrange("(kc p) n -> p kc n", p=128))
                nc.sync.dma_start(wks[:], wk.rearrange("(kc p) n -> p kc n", p=128))
                nc.sync.dma_start(wvs[:], wv.rearrange("(kc p) n -> p kc n", p=128))

                for t in range(N_TC):
                    xt = xp.tile([128, N_CC, TC], F32R, tag="xt")
                    nc.sync.dma_start(
                        xt[:],
                        xT[:, t * TC:(t + 1) * TC].rearrange(
                            "(kc p) n -> p kc n", p=128
                        ),
                    )
                    for oc in range(N_GCB):      # QT and KT column blocks
                        for w_s, dst, scale in ((wqs, qt, 0.125), (wks, kt, None)):
                            ps = pjp.tile([128, TC], F32, tag="pj")
                            for kc in range(N_CC):
                                nc.tensor.matmul(
                                    ps[:],
                                    w_s[:, kc, oc * 128:(oc + 1) * 128],
                                    xt[:, kc, :],
                                    start=(kc == 0),
                                    stop=(kc == N_CC - 1),
                                )
                            dslc = dst[:, oc, t * TC:(t + 1) * TC]
                            if scale is None:
                                nc.vector.tensor_copy(dslc, ps[:])
                            else:
                                nc.vector.tensor_scalar_mul(dslc, ps[:], scale)
                    for tb in range(TC // 128):  # V token blocks
                        ps = pjp.tile([128, GC], F32, tag="pj")
                        for kc in range(N_CC):
                            nc.tensor.matmul(
                                ps[:],
                                xt[:, kc, tb * 128:(tb + 1) * 128],
                                wvs[:, kc, :],
                                start=(kc == 0),
                                stop=(kc == N_CC - 1),
                            )
                        nc.vector.tensor_copy(
                            vaug[:, t * 4 + tb, :, :Dh],
                            ps.rearrange("p (h d) -> p h d", h=HPG),
                        )

            # ---------------- phase 3: attention -----------------------
            with (
                tc.tile_pool(name="probs", bufs=6) as prp,
                tc.tile_pool(name="norm", bufs=4) as nrm,
                tc.tile_pool(name="sc_psum", bufs=3, space="PSUM") as scp,
                tc.tile_pool(name="av_psum", bufs=4, space="PSUM") as avp,
                tc.tile_pool(name="bc_psum", bufs=1, space="PSUM") as bcp,
            ):
                for p in range(HPG // 2):        # head pairs: rows 0:64 / 64:128
                    for j in range(N_TC):        # query chunk
                        qslc = slice(j * TC, (j + 1) * TC)
                        avs = [
                            avp.tile([Dh + 1, TC], F32, tag="av", name=f"av{i}")
                            for i in range(2)
                        ]
                        nkb = 4 * j + 4
                        for kb in range(nkb):
                            for half in range(2):
                                h = 2 * p + half
                                p0 = half * Dh
                                sc = scp.tile([128, TC], F32, tag="sc")
                                nc.tensor.matmul(
                                    sc[:],
                                    kt[p0:p0 + Dh, p, kb * KB:(kb + 1) * KB],
                                    qt[p0:p0 + Dh, p, qslc],
                                    start=True,
                                    stop=True,
                                )
                                pr = prp.tile([128, TC], BF16, tag="pr")
                                nc.scalar.activation(
                                    pr[:], sc[:], mybir.ActivationFunctionType.Exp
                                )
                                m = kb - 4 * j
                                if m >= 0:
                                    nc.vector.tensor_mul(
                                        pr[:], pr[:], msk[:, m, :]
                                    )
                                nc.tensor.matmul(
                                    avs[half][:],
                                    vaug[:, kb, h, :],
                                    pr[:],
                                    start=(kb == 0),
                                    stop=(kb == nkb - 1),
                                )
                        for half in range(2):
                            h = 2 * p + half
                            p0 = half * Dh
                            rr = nrm.tile([1, TC], F32R, tag="rr")
                            with nc.allow_low_precision(
                                reason="fp32r reciprocal row feeds bcast matmul"
                            ):
                                nc.vector.reciprocal(
                                    rr[:], avs[half][Dh:Dh + 1, :]
                                )
                            bc = bcp.tile([Dh, TC], F32, tag="bc")
                            nc.tensor.matmul(
                                bc[:], ones[:], rr[:], start=True, stop=True
                            )
                            rb = nrm.tile([Dh, TC], F32, tag="rb")
                            nc.vector.tensor_copy(rb[:], bc[:])
                            nc.vector.tensor_mul(
                                aot[p0:p0 + Dh, p, qslc], avs[half][:Dh, :], rb[:]
                            )

            # ---------------- phase 4: output projection ----------------
            with (
                tc.tile_pool(name="outs", bufs=3) as otp,
                tc.tile_pool(name="wo_pool", bufs=1) as wop,
                tc.tile_pool(name="out_psum", bufs=2, space="PSUM") as opp,
            ):
                wos = wop.tile([128, N_GCB, C], F32R)
                nc.sync.dma_start(wos[:], wo.rearrange("(cb p) n -> p cb n", p=128))
                for tb in range(N_KB):
                    ot = otp.tile([128, C], F32, tag="ot")
                    for oc in range(C // TC):
                        ps = opp.tile([128, TC], F32, tag="op")
                        for cc in range(N_GCB):
                            nc.tensor.matmul(
                                ps[:],
                                aot[:, cc, tb * 128:(tb + 1) * 128],
                                wos[:, cc, oc * TC:(oc + 1) * TC],
                                start=(cc == 0),
                                stop=(cc == N_GCB - 1),
                            )
                        nc.vector.tensor_copy(ot[:, oc * TC:(oc + 1) * TC], ps[:])
                    nc.sync.dma_start(out[tb * 128:(tb + 1) * 128, :], ot[:])

    nc.compile()
    return nc


_CACHE = {}


def _make_masks():
    m = np.zeros((4, KB, TC), np.float32)
    for i in range(4):
        for dk in range(KB):
            m[i, dk, KB * i + dk:] = 1.0
    return m.astype(ml_dtypes.bfloat16)


def make_in_maps(x, W_qkv, W_out):
    masks = _make_masks()
    in_maps = []
    for core in range(N_CORES):
        b, g = divmod(core, G)
        cs = slice(g * GC, (g + 1) * GC)
        in_maps.append({
            "xT": np.ascontiguousarray(x[b].T),
            "wq": np.ascontiguousarray(W_qkv[:, cs]),
            "wk": np.ascontiguousarray(W_qkv[:, C + g * GC:C + (g + 1) * GC]),
            "wv": np.ascontiguousarray(W_qkv[:, 2 * C + g * GC:2 * C + (g + 1) * GC]),
            "wo": np.ascontiguousarray(W_out[cs, :]),
            "masks": masks,
            "ones": np.ones((1, Dh), np.float32),
        })
    return in_maps


def kernel(x, W_qkv, W_out):
    x = np.ascontiguousarray(np.asarray(x, dtype=np.float32))
    W_qkv = np.asarray(W_qkv, dtype=np.float32)
    W_out = np.asarray(W_out, dtype=np.float32)

    if "nc" not in _CACHE:
        _CACHE["nc"] = build_program()
    nc = _CACHE["nc"]

    in_maps = make_in_maps(x, W_qkv, W_out)
    res = bass_utils.run_bass_kernel_spmd(nc, in_maps, core_ids=list(range(N_CORES)))

    out = np.empty((B, T, C), np.float32)
    for b in range(B):
        out[b] = res.results[G * b]["out"]
        for g in range(1, G):
            out[b] += res.results[G * b + g]["out"]
    return out


# revision 24
# speedup vs baseline: 1.6068x; 1.6068x over previous
"""Multi-head causal attention on 8 Trainium2 NeuronCores.

Sharding: data-parallel over batch (4) x tensor-parallel over heads (2 groups
of 8 heads). Each core computes a partial output [T, C] for one batch element
using its 8 heads; the host sums the two partials per batch element (the
"all-reduce after out_proj" done during unshard).

Per-core algorithm (all layouts chosen so no on-device transposes are needed):
  inputs: xT [C, T] (x[b] transposed on host), Wq/Wk/Wv [C, 512], Wo [512, C],
          causal multiplicative masks [4, 128, 512] (bf16).
  QT = Wq^T @ x^T * 1/sqrt(Dh)  [512, T]   (lhsT = Wq chunk, rhs = xT chunk)
  KT = Wk^T @ x^T               [512, T]
  V  = x @ Wv                   [T, 512]   (lhsT = xT chunk, rhs = Wv)
       stored ones-augmented as V_aug [T, 8 heads, 65] bf16 (col 64 = 1.0)
  per head h, query-chunk j (512 wide), key-block kb (128 wide, causal only):
     sT  = K_h[kb]^T @ Q_h[:, j]           [128, 512] PSUM  (fp32r matmul)
     p   = exp(sT)                         bf16 (skip-max softmax: |s| < ~9)
     p  *= mask                            (diagonal blocks only)
     av += V_aug[kb, h]^T @ p              [65, 512] PSUM; row 64 = denom
  attn_outT[h, :, j] = av[:64] * (1/denom broadcast via ones-matmul)
  out[tb] = attn_outT[:, tb]^T @ Wo        [128, 1024] -> partial output
"""

import numpy as np
import ml_dtypes

import concourse.bass as bass
import concourse.bacc as bacc
import concourse.mybir as mybir
import concourse.tile as tile
from concourse import bass_utils

F32 = mybir.dt.float32
F32R = mybir.dt.float32r
BF16 = mybir.dt.bfloat16

B, T, C = 4, 2048, 1024
H, Dh = 16, 64
G = 2                 # head groups (tensor parallel)
HPG = H // G          # heads per group
GC = HPG * Dh         # group channels = 512
N_CORES = 8
TC = 512              # token chunk (phase 2 and query chunks)
KB = 128              # key block
N_TC = T // TC        # 4
N_KB = T // KB        # 16
N_CC = C // 128       # contraction chunks over C = 8
N_GCB = GC // 128     # chan blocks in a group = 4


def build_program():
    nc = bacc.Bacc("TRN2", target_bir_lowering=False, debug=False)

    xT = nc.dram_tensor("xT", [C, T], F32R, kind="ExternalInput").ap()
    wq = nc.dram_tensor("wq", [C, GC], F32R, kind="ExternalInput").ap()
    wk = nc.dram_tensor("wk", [C, GC], F32R, kind="ExternalInput").ap()
    wv = nc.dram_tensor("wv", [C, GC], F32R, kind="ExternalInput").ap()
    wo = nc.dram_tensor("wo", [GC, C], F32R, kind="ExternalInput").ap()
    masks = nc.dram_tensor("masks", [4, KB, TC], BF16, kind="ExternalInput").ap()
    ones_in = nc.dram_tensor("ones", [1, Dh], F32R, kind="ExternalInput").ap()
    sel_in = nc.dram_tensor("sel", [32, 32 * Dh], F32R, kind="ExternalInput").ap()
    out = nc.dram_tensor("out", [T, C], F32, kind="ExternalOutput").ap()

    with tile.TileContext(nc) as tc:
        with tc.tile_pool(name="persist", bufs=1) as pp:
            qt = pp.tile([128, N_GCB, T], F32R)        # QT (chan%128, chan//128, tok)
            kt = pp.tile([128, N_GCB, T], F32R)
            vaug = pp.tile([128, N_KB, HPG, Dh + 1], BF16)
            aot = pp.tile([128, N_GCB, T], F32R)       # attn_outT
            msk = pp.tile([128, 4, TC], BF16)
            ones = pp.tile([1, Dh], F32R)
            sel = pp.tile([32, 32 * Dh], F32R)

            nc.sync.dma_start(msk[:], masks.rearrange("m p n -> p m n"))
            nc.sync.dma_start(ones[:], ones_in)
            nc.sync.dma_start(sel[:], sel_in)
            nc.vector.memset(vaug[:, :, :, Dh:], 1.0)

            # ---------------- phase 2: qkv projections -----------------
            with (
                tc.tile_pool(name="wq_pool", bufs=1) as wqp,
                tc.tile_pool(name="x_pool", bufs=2) as xp,
                tc.tile_pool(name="proj_psum", bufs=4, space="PSUM") as pjp,
            ):
                wqs = wqp.tile([128, N_CC, GC], F32R, tag="wq")
                wks = wqp.tile([128, N_CC, GC], F32R, tag="wk")
                wvs = wqp.tile([128, N_CC, GC], F32R, tag="wv")
                nc.sync.dma_start(wqs[:], wq.rearrange("(kc p) n -> p kc n", p=128))
                nc.sync.dma_start(wks[:], wk.rearrange("(kc p) n -> p kc n", p=128))
                nc.sync.dma_start(wvs[:], wv.rearrange("(kc p) n -> p kc n", p=128))

                for t in range(N_TC):
                    xt = xp.tile([128, N_CC, TC], F32R, tag="xt")
                    nc.sync.dma_start(
                        xt[:],
                        xT[:, t * TC:(t + 1) * TC].rearrange(
                            "(kc p) n -> p kc n", p=128
                        ),
                    )
                    for oc in range(N_GCB):      # QT and KT column blocks
                        for w_s, dst, scale in ((wqs, qt, 0.125), (wks, kt, None)):
                            ps = pjp.tile([128, TC], F32, tag="pj")
                            for kc in range(N_CC):
                                nc.tensor.matmul(
                                    ps[:],
                                    w_s[:, kc, oc * 128:(oc + 1) * 128],
                                    xt[:, kc, :],
                                    start=(kc == 0),
                                    stop=(kc == N_CC - 1),
                                )
                            dslc = dst[:, oc, t * TC:(t + 1) * TC]
                            if scale is None:
                                nc.vector.tensor_copy(dslc, ps[:])
                            else:
                                nc.vector.tensor_scalar_mul(dslc, ps[:], scale)
                    for tb in range(TC // 128):  # V token blocks
                        ps = pjp.tile([128, GC], F32, tag="pj")
                        for kc in range(N_CC):
                            nc.tensor.matmul(
                                ps[:],
                                xt[:, kc, tb * 128:(tb + 1) * 128],
                                wvs[:, kc, :],
                                start=(kc == 0),
                                stop=(kc == N_CC - 1),
                            )
                        nc.vector.tensor_copy(
                            vaug[:, t * 4 + tb, :, :Dh],
                            ps.rearrange("p (h d) -> p h d", h=HPG),
                        )

            # ---------------- phase 3: attention -----------------------
            # Softmax denominator rows staged for batched normalization.
            # Engine APs may only start at partitions {0,32,64}, so slot s
            # lives at (partition 32*(s//11), column s%11); a DMA later
            # compacts the slots into a [32, TC] tile for one reciprocal.
            lctx = tc.tile_pool(name="ph3_long", bufs=1)
            lp = lctx.__enter__()
            dens = lp.tile([65, 11, TC], F32)
            with (
                tc.tile_pool(name="probs", bufs=4) as prp,
                tc.tile_pool(name="sc_psum", bufs=2, space="PSUM") as scp,
                tc.tile_pool(name="av_psum", bufs=4, space="PSUM") as avp,
            ):
                for j in range(N_TC):            # query chunk
                    for p in range(HPG // 2):    # head pairs: rows 0:64 / 64:128
                        qslc = slice(j * TC, (j + 1) * TC)
                        avs = [
                            avp.tile([Dh + 1, TC], F32, tag="av", name=f"av{i}")
                            for i in range(2)
                        ]
                        nkb = 4 * j + 4
                        for kb in range(nkb):
                            # both heads' score tiles side by side in one
                            # 2-bank PSUM tile -> single exp op per kb
                            sc = scp.tile([128, 2 * TC], F32, tag="sc")
                            for half in range(2):
                                p0 = half * Dh
                                nc.tensor.matmul(
                                    sc[:, half * TC:(half + 1) * TC],
                                    kt[p0:p0 + Dh, p, kb * KB:(kb + 1) * KB],
                                    qt[p0:p0 + Dh, p, qslc],
                                    start=True,
                                    stop=True,
                                )
                            pr = prp.tile([128, 2 * TC], BF16, tag="pr")
                            nc.scalar.activation(
                                pr[:], sc[:], mybir.ActivationFunctionType.Exp
                            )
                            m = kb - 4 * j
                            if m >= 0:
                                for half in range(2):
                                    nc.vector.tensor_mul(
                                        pr[:, half * TC:(half + 1) * TC],
                                        pr[:, half * TC:(half + 1) * TC],
                                        msk[:, m, :],
                                    )
                            for half in range(2):
                                nc.tensor.matmul(
                                    avs[half][:],
                                    vaug[:, kb, 2 * p + half, :],
                                    pr[:, half * TC:(half + 1) * TC],
                                    start=(kb == 0),
                                    stop=(kb == nkb - 1),
                                )
                        for half in range(2):
                            p0 = half * Dh
                            idx = (j * 4 + p) * 2 + half
                            nc.vector.tensor_copy(
                                aot[p0:p0 + Dh, p, qslc], avs[half][:Dh, :]
                            )
                            db, dc = 32 * (idx // 11), idx % 11
                            nc.vector.tensor_copy(
                                dens[db:db + 1, dc, :], avs[half][Dh:Dh + 1, :]
                            )

            # batched normalization: one reciprocal, then per-slot
            # selection-matmul broadcast + in-place multiply on aot
            with (
                tc.tile_pool(name="bc_psum", bufs=2, space="PSUM") as bcp,
            ):
                dcomp = lp.tile([32, TC], F32)
                for b in range(3):
                    lo, n = 11 * b, min(11, 32 - 11 * b)
                    nc.sync.dma_start(
                        dcomp[lo:lo + n, :], dens[32 * b:32 * b + 1, :n, :]
                    )
                rec = lp.tile([32, TC], F32R)
                with nc.allow_low_precision(
                    reason="fp32r reciprocal feeds bcast matmul"
                ):
                    nc.vector.reciprocal(rec[:], dcomp[:])
                for j in range(N_TC):
                    for p in range(HPG // 2):
                        for half in range(2):
                            p0 = half * Dh
                            idx = (j * 4 + p) * 2 + half
                            qslc = slice(j * TC, (j + 1) * TC)
                            bc = bcp.tile([Dh, TC], F32, tag="bc")
                            nc.tensor.matmul(
                                bc[:],
                                sel[:, idx * Dh:(idx + 1) * Dh],
                                rec[:],
                                start=True, stop=True,
                            )
                            nc.vector.tensor_mul(
                                aot[p0:p0 + Dh, p, qslc],
                                aot[p0:p0 + Dh, p, qslc],
                                bc[:],
                            )
            lctx.__exit__(None, None, None)

            # ---------------- phase 4: output projection ----------------
            with (
                tc.tile_pool(name="outs", bufs=3) as otp,
                tc.tile_pool(name="wo_pool", bufs=1) as wop,
                tc.tile_pool(name="out_psum", bufs=2, space="PSUM") as opp,
            ):
                wos = wop.tile([128, N_GCB, C], F32R)
                nc.sync.dma_start(wos[:], wo.rearrange("(cb p) n -> p cb n", p=128))
                for tb in range(N_KB):
                    ot = otp.tile([128, C], F32, tag="ot")
                    for oc in range(C // TC):
                        ps = opp.tile([128, TC], F32, tag="op")
                        for cc in range(N_GCB):
                            nc.tensor.matmul(
                                ps[:],
                                aot[:, cc, tb * 128:(tb + 1) * 128],
                                wos[:, cc, oc * TC:(oc + 1) * TC],
                                start=(cc == 0),
                                stop=(cc == N_GCB - 1),
                            )
                        nc.vector.tensor_copy(ot[:, oc * TC:(oc + 1) * TC], ps[:])
                    nc.sync.dma_start(out[tb * 128:(tb + 1) * 128, :], ot[:])

    nc.compile()
    return nc


_CACHE = {}


def _make_masks():
    m = np.zeros((4, KB, TC), np.float32)
    for i in range(4):
        for dk in range(KB):
            m[i, dk, KB * i + dk:] = 1.0
    return m.astype(ml_dtypes.bfloat16)


def _make_sel():
    s = np.zeros((32, 32 * Dh), np.float32)
    for i in range(32):
        s[i, i * Dh:(i + 1) * Dh] = 1.0
    return s


def make_in_maps(x, W_qkv, W_out):
    masks = _make_masks()
    in_maps = []
    for core in range(N_CORES):
        b, g = divmod(core, G)
        cs = slice(g * GC, (g + 1) * GC)
        in_maps.append({
            "xT": np.ascontiguousarray(x[b].T),
            "wq": np.ascontiguousarray(W_qkv[:, cs]),
            "wk": np.ascontiguousarray(W_qkv[:, C + g * GC:C + (g + 1) * GC]),
            "wv": np.ascontiguousarray(W_qkv[:, 2 * C + g * GC:2 * C + (g + 1) * GC]),
            "wo": np.ascontiguousarray(W_out[cs, :]),
            "masks": masks,
            "ones": np.ones((1, Dh), np.float32),
            "sel": _make_sel(),
        })
    return in_maps


def kernel(x, W_qkv, W_out):
    x = np.ascontiguousarray(np.asarray(x, dtype=np.float32))
    W_qkv = np.asarray(W_qkv, dtype=np.float32)
    W_out = np.asarray(W_out, dtype=np.float32)

    if "nc" not in _CACHE:
        _CACHE["nc"] = build_program()
    nc = _CACHE["nc"]

    in_maps = make_in_maps(x, W_qkv, W_out)
    res = bass_utils.run_bass_kernel_spmd(nc, in_maps, core_ids=list(range(N_CORES)))

    out = np.empty((B, T, C), np.float32)
    for b in range(B):
        out[b] = res.results[G * b]["out"]
        for g in range(1, G):
            out[b] += res.results[G * b + g]["out"]
    return out
